# revision 1
# baseline (speedup 1.0000x reference)
"""Causal MHA (B=4, L=2048, D=1024, H=16) on 8 NeuronCores.

Sharding: core c -> (batch b = c//2, head-group g = c%2). Data-parallel over
the 4 batches, tensor-parallel over heads (8 heads per core): wq/wk/wv
column-parallel, wo row-parallel. Each core returns a partial [L, D] output;
the host sums the two head-group partials per batch and adds wo_b.

Per-core device kernel (all matmuls fp32r: 1 cyc/row at N>=256, ~1.5e-4 rel):
  A) QT = (wq_g*0.125) @ q_b.T + bq  -> [512, 2048] SBUF (head dims on parts)
     KT likewise (unscaled).  V_aug = q_b @ wv_aug.T + vb -> [2048, 520] DRAM
     (per head: 64 dims + a ones column -> fused softmax denominator).
  B) per head h, per 512-wide q-slice: S.T[keys,q] = KT_h.T-slice @ QT_h
     (causal-trimmed N), exp on ScalarE, tri-mask on the diagonal 128-block,
     AV: psum[65, q] += V_aug_h[kb].T @ P.T  (row 64 = denominator).
     Normalize rows 0..63 by 1/denom (DVE recip + GpSimd partition_broadcast
     + DVE mul) -> ctxT [512, 2048] spilled to DRAM.
  C) out_partial[t, :] = sum_c ctxT[c, t-tile].T @ woT[c] -> [2048, 1024] f32.
"""

import numpy as np

import concourse.bacc as bacc
import concourse.bass as bass
import concourse.mybir as mybir
import concourse.tile as tile
from concourse.bass_utils import run_bass_kernel_spmd

F32 = mybir.dt.float32
F32R = mybir.dt.float32r

B, L, D, H, DK = 4, 2048, 1024, 16, 64
HD = 8            # heads per core
GW = 512          # head-group width (8 heads * 64)
AUGW = HD * (DK + 1)  # 520: per head 64 dims + ones col (ones LAST per head)
NCH = D // 128    # 8 contraction chunks
QS = 512          # q-slice width in attention
NQS = L // QS     # 4
NKB = L // 128    # 16 key blocks
NTT = L // 128    # 16 token tiles


def _r(ap):
    return ap


def _build_nc(dbg=False, phases="ABC"):
    nc = bacc.Bacc("TRN2", target_bir_lowering=False, debug=False, num_devices=8)

    xq = nc.dram_tensor("xq", [D, L], F32R, kind="ExternalInput").ap()
    xk = nc.dram_tensor("xk", [D, L], F32R, kind="ExternalInput").ap()
    xv = nc.dram_tensor("xv", [D, L], F32R, kind="ExternalInput").ap()
    wq = nc.dram_tensor("wq", [D, GW], F32R, kind="ExternalInput").ap()
    wk = nc.dram_tensor("wk", [D, GW], F32R, kind="ExternalInput").ap()
    wv = nc.dram_tensor("wv", [D, AUGW], F32R, kind="ExternalInput").ap()
    wo = nc.dram_tensor("wo", [GW, D], F32R, kind="ExternalInput").ap()
    bq = nc.dram_tensor("bq", [128, 4], F32, kind="ExternalInput").ap()
    bk = nc.dram_tensor("bk", [128, 4], F32, kind="ExternalInput").ap()
    vb = nc.dram_tensor("vb", [AUGW], F32, kind="ExternalInput").ap()
    msk = nc.dram_tensor("msk", [128, 128], F32, kind="ExternalInput").ap()
    outp = nc.dram_tensor("outp", [L, D], F32, kind="ExternalOutput").ap()
    if dbg:
        qt_dbg = nc.dram_tensor("qt_dbg", [128, 4 * L], F32, kind="ExternalOutput").ap()
        kt_dbg = nc.dram_tensor("kt_dbg", [128, 4 * L], F32, kind="ExternalOutput").ap()
        vg_dbg = nc.dram_tensor("vg_dbg", [L, AUGW], F32, kind="ExternalOutput").ap()
        ctx_dbg = nc.dram_tensor("ctx_dbg", [GW, L], F32, kind="ExternalOutput").ap()

    with tile.TileContext(nc) as tc:
        with (
            tc.tile_pool(name="persist", bufs=1) as persist,
            tc.tile_pool(name="xin", bufs=10) as xinp,
            tc.tile_pool(name="work", bufs=4) as workp,
            tc.tile_pool(name="pt", bufs=5) as ptp,
            tc.tile_pool(name="vh", bufs=2) as vhp,
            tc.tile_pool(name="small", bufs=4) as smallp,
            tc.tile_pool(name="outs", bufs=3) as outsp,
            tc.tile_pool(name="psA", bufs=6, space="PSUM") as psA,
            tc.tile_pool(name="psC", bufs=2, space="PSUM") as psC,
            tc.tile_pool(name="dram", bufs=1, space="DRAM") as dramp,
            tc.tile_pool(name="dnb", bufs=4, space="DRAM") as dnbp,
        ):
            # ---- persistent SBUF ----
            wq_s = persist.tile([128, NCH, GW], F32R, tag="wq")
            wk_s = persist.tile([128, NCH, GW], F32R, tag="wk")
            wv_s = persist.tile([128, NCH, AUGW], F32R, tag="wv")
            wo_s = persist.tile([128, 4, D], F32R, tag="wo")
            qt_s = persist.tile([128, 4, L], F32R, tag="qt")
            kt_s = persist.tile([128, 4, L], F32R, tag="kt")
            bq_s = persist.tile([128, 4], F32, tag="bq")
            bk_s = persist.tile([128, 4], F32, tag="bk")
            vb_s = persist.tile([128, AUGW], F32, tag="vb")
            msk_s = persist.tile([128, 128], F32, tag="msk")

            vg_d = dramp.tile([L, AUGW], F32R, tag="vg")
            ctx_d = dramp.tile([GW, L], F32R, tag="ctx")

            for c in range(NCH):
                nc.sync.dma_start(wq_s[:, c, :], wq[c * 128:(c + 1) * 128, :])
                nc.sync.dma_start(wk_s[:, c, :], wk[c * 128:(c + 1) * 128, :])
                nc.sync.dma_start(wv_s[:, c, :], wv[c * 128:(c + 1) * 128, :])
            for c in range(4):
                nc.sync.dma_start(wo_s[:, c, :], wo[c * 128:(c + 1) * 128, :])
            nc.sync.dma_start(bq_s[:, :], bq[:, :])
            nc.sync.dma_start(bk_s[:, :], bk[:, :])
            nc.sync.dma_start(msk_s[:, :], msk[:, :])
            vb_bcast = bass.AP(tensor=vb.tensor, offset=vb.offset,
                               ap=[[0, 128], [1, AUGW]])
            nc.gpsimd.dma_start(vb_s[:, :], vb_bcast)

            # ---- phase A: projections ----
            for n in range(4):  # 512-token slice
                for (src, w_s, dst, b_s) in ((xq, wq_s, qt_s, bq_s),
                                             (xk, wk_s, kt_s, bk_s)):
                    xt = []
                    for c in range(NCH):
                        t = xinp.tile([128, 512], F32R, tag="xin")
                        nc.sync.dma_start(
                            t[:, :], src[c * 128:(c + 1) * 128,
                                         n * 512:(n + 1) * 512])
                        xt.append(t)
                    pss = [psA.tile([128, 512], F32, tag="ps", name=f"psA{i}") for i in range(4)]
                    for c in range(NCH):
                        for m in range(4):
                            nc.tensor.matmul(
                                pss[m][:, :],
                                _r(w_s[:, c, m * 128:(m + 1) * 128]),
                                _r(xt[c][:, :]),
                                start=(c == 0), stop=(c == NCH - 1))
                    for m in range(4):
                        nc.vector.tensor_scalar_add(
                            dst[:, m, n * 512:(n + 1) * 512],
                            pss[m][:, :], b_s[:, m:m + 1])
                # V_aug
                xt = []
                for c in range(NCH):
                    t = xinp.tile([128, 512], F32R, tag="xin")
                    nc.sync.dma_start(
                        t[:, :], xv[c * 128:(c + 1) * 128,
                                    n * 512:(n + 1) * 512])
                    xt.append(t)
                for tt in range(4):  # token tile within slice
                    for hf in range(2):
                        ps = psA.tile([128, 260], F32, tag="ps")
                        for c in range(NCH):
                            nc.tensor.matmul(
                                ps[:, :],
                                _r(xt[c][:, tt * 128:(tt + 1) * 128]),
                                _r(wv_s[:, c, hf * 260:(hf + 1) * 260]),
                                start=(c == 0), stop=(c == NCH - 1))
                        vst = workp.tile([128, 260], F32R, tag="vst")
                        nc.vector.tensor_add(
                            vst[:, :], ps[:, :],
                            vb_s[:, hf * 260:(hf + 1) * 260])
                        nc.sync.dma_start(
                            vg_d[(n * 4 + tt) * 128:(n * 4 + tt + 1) * 128,
                                 hf * 260:(hf + 1) * 260],
                            vst[:, :])

            # ---- phase B: attention, two heads interleaved ----
            def emit_head_qs(h, vh, qs):
                po = (h % 2) * 64   # partition offset inside chunk
                mc = h // 2         # chunk index for this head
                cps = psC.tile([DK + 1, QS], F32, tag="cps", name=f"cps{h}_{qs}")
                nkb = 4 * qs + 4
                pts = [None] * nkb
                c0s = [None] * nkb

                def emit_st(kb):
                    col0 = max(0, kb * 128 - qs * QS)
                    sp = psA.tile([128, QS], F32, tag="ps", name=f"sp{h}_{qs}_{kb}")
                    nc.tensor.matmul(
                        sp[:, col0:],
                        _r(kt_s[po:po + 64, mc, kb * 128:(kb + 1) * 128]),
                        _r(qt_s[po:po + 64, mc,
                                qs * QS + col0:(qs + 1) * QS]),
                        start=True, stop=True)
                    pt = ptp.tile([128, QS], F32R, tag="pt", name=f"pt{h}_{qs}_{kb}")
                    nc.scalar.activation(
                        pt[:, col0:], sp[:, col0:],
                        func=mybir.ActivationFunctionType.Exp)
                    if col0 > 0 or kb == 4 * qs:
                        nc.vector.tensor_mul(
                            pt[:, col0:col0 + 128],
                            pt[:, col0:col0 + 128], msk_s[:, :])
                    pts[kb] = pt
                    c0s[kb] = col0

                def emit_av(kb):
                    col0 = c0s[kb]
                    nc.tensor.matmul(
                        cps[:, col0:],
                        _r(vh[:, kb, :]),
                        _r(pts[kb][:, col0:]),
                        start=(kb == 0), stop=(kb == nkb - 1))

                emit_st(0)
                emit_st(1)
                for kb in range(2, nkb):
                    emit_st(kb)
                    emit_av(kb - 2)
                emit_av(nkb - 2)
                emit_av(nkb - 1)

                rc = smallp.tile([128, QS], F32, tag="rc", name=f"rc{h}_{qs}")
                nc.vector.reciprocal(rc[64:65, :], cps[64:65, :])
                dn = dnbp.tile([1, QS], F32, tag="dn", name=f"dn{h}_{qs}")
                nc.sync.dma_start(dn[0:1, :], rc[64:65, :])
                bc = smallp.tile([64, QS], F32, tag="bc", name=f"bc{h}_{qs}")
                nc.sync.dma_start(bc[:, :],
                                  dn[0:1, :].partition_broadcast(64))
                co = workp.tile([64, QS], F32R, tag="co", name=f"co{h}_{qs}")
                nc.vector.tensor_mul(co[:, :], cps[0:64, :], bc[:, :])
                nc.sync.dma_start(
                    ctx_d[h * 64:(h + 1) * 64, qs * QS:(qs + 1) * QS],
                    co[:, :])

            for hp in (range(HD // 2) if "B" in phases else []):
                h0, h1 = 2 * hp, 2 * hp + 1
                vhs = []
                for h in (h0, h1):
                    vh = vhp.tile([128, NKB, DK + 1], F32R, tag="vh",
                                  name=f"vh{h}")
                    nc.sync.dma_start(
                        vh[:, :, :],
                        vg_d[:, h * 65:(h + 1) * 65].rearrange(
                            "(t p) a -> p t a", p=128))
                    vhs.append(vh)
                for qs in range(NQS):
                    emit_head_qs(h0, vhs[0], qs)
                    emit_head_qs(h1, vhs[1], qs)

            # ---- phase C: output projection ----
            for t in (range(NTT) if "C" in phases else []):
                cts = []
                for c in range(4):
                    ct = workp.tile([128, 128], F32R, tag="ct", bufs=8)
                    nc.sync.dma_start(
                        ct[:, :], ctx_d[c * 128:(c + 1) * 128,
                                        t * 128:(t + 1) * 128])
                    cts.append(ct)
                pss = [psA.tile([128, 512], F32, tag="ps", name=f"psC{i}") for i in range(2)]
                for c in range(4):
                    for n2 in range(2):
                        nc.tensor.matmul(
                            pss[n2][:, :], _r(cts[c][:, :]),
                            _r(wo_s[:, c, n2 * 512:(n2 + 1) * 512]),
                            start=(c == 0), stop=(c == 3))
                for n2 in range(2):
                    ot = outsp.tile([128, 512], F32, tag="ot")
                    nc.vector.tensor_copy(ot[:, :], pss[n2][:, :])
                    nc.sync.dma_start(
                        outp[t * 128:(t + 1) * 128,
                             n2 * 512:(n2 + 1) * 512], ot[:, :])

            if dbg:
                nc.sync.dma_start(qt_dbg[:, :], qt_s[:, :, :].bitcast(F32))
                nc.sync.dma_start(kt_dbg[:, :], kt_s[:, :, :].bitcast(F32))
                nc.sync.dma_start(vg_dbg[:, :], vg_d[:, :].bitcast(F32))
                nc.sync.dma_start(ctx_dbg[:, :], ctx_d[:, :].bitcast(F32))

    nc.compile()
    return nc


_NC = None
LAST_RESULTS = None


def kernel(**inputs):
    global _NC, LAST_RESULTS
    import os
    if _NC is None:
        _NC = _build_nc()

    f = lambda a: np.asarray(a, dtype=np.float32)
    q, k, v = f(inputs["q"]), f(inputs["k"]), f(inputs["v"])
    wq_w, wq_b = f(inputs["wq_w"]), f(inputs["wq_b"])
    wk_w, wk_b = f(inputs["wk_w"]), f(inputs["wk_b"])
    wv_w, wv_b = f(inputs["wv_w"]), f(inputs["wv_b"])
    wo_w, wo_b = f(inputs["wo_w"]), f(inputs["wo_b"])

    msk = np.ascontiguousarray(
        (np.arange(128)[None, :] >= np.arange(128)[:, None]).astype(np.float32))

    gmaps = []
    for g in range(2):
        sl = slice(g * GW, (g + 1) * GW)
        wqT = np.ascontiguousarray((wq_w[sl] * 0.125).T)
        wkT = np.ascontiguousarray(wk_w[sl].T)
        wvT = np.zeros((D, AUGW), np.float32)
        vbias = np.zeros((AUGW,), np.float32)
        for h in range(HD):
            wvT[:, h * 65:h * 65 + 64] = wv_w[g * GW + h * 64:
                                              g * GW + (h + 1) * 64].T
            vbias[h * 65:h * 65 + 64] = wv_b[g * GW + h * 64:
                                             g * GW + (h + 1) * 64]
            vbias[h * 65 + 64] = 1.0
        woT = np.ascontiguousarray(wo_w[:, sl].T)
        bqT = np.ascontiguousarray(
            (wq_b[sl] * 0.125).reshape(4, 128).T)
        bkT = np.ascontiguousarray(wk_b[sl].reshape(4, 128).T)
        gmaps.append(dict(wq=wqT, wk=wkT, wv=wvT, wo=woT, bq=bqT, bk=bkT,
                          vb=vbias, msk=msk))

    bmaps = []
    for b in range(B):
        bmaps.append(dict(
            xq=np.ascontiguousarray(q[b].T),
            xk=np.ascontiguousarray(k[b].T),
            xv=np.ascontiguousarray(v[b].T)))

    in_maps = [dict(**bmaps[c // 2], **gmaps[c % 2]) for c in range(8)]

    trace = bool(int(os.environ.get("KERNEL_TRACE", "0")))
    res = run_bass_kernel_spmd(_NC, in_maps, list(range(8)), trace=trace)
    LAST_RESULTS = res

    out = np.empty((B, L, D), np.float32)
    for b in range(B):
        out[b] = (res.results[2 * b]["outp"] + res.results[2 * b + 1]["outp"]
                  + wo_b[None, :])
    return out



# revision 3
# speedup vs baseline: 1.7271x; 1.7271x over previous
"""Causal MHA (B=4, L=2048, D=1024, H=16) on 8 NeuronCores — v2, all bf16.

Sharding: core c -> (batch b = c//2, head-group g = c%2); 8 heads per core.
Host sums the two head-group partial outputs per batch and adds wo_b.

Per-core structure (single Bass module, software-pipelined emission):
  A(n): per 512-token slice n: QT/KT = w @ x chunks -> psum -> +bias -> SBUF
        bf16 [128, 4, L] (head-dim on partitions).  V2 = x.T @ wv -> psum
        [tok,4,64] -> +bias -> SBUF v2[128 tok, kb, 8 heads, 128] where cols
        64:128 of each head slot are constant 1.0 (preset once): the AV
        matmul then yields the softmax denominator replicated on 64
        partitions for free.
  B(h, qs): score S.T[keys,q] = KT_h.T @ QT_h (causal-trimmed), exp on
        ACT -> pt bf16, tri-mask on diagonal blocks (DVE), AV psum
        [128, q] += v2[kb,h].T @ pt: rows 0:64 ctx, 64:128 denominator.
        ctx = rows/denominator (one DVE divide) -> SBUF ctx bf16.
  C(t): out[tok128, 1024] = sum_c ctx[c].T @ wo[c] -> psum -> bf16 -> DRAM.
  A(n+1) and C(t) matmuls are interleaved into B's emission as filler so
  the PE never waits on the exp (ACT) pipeline.
"""

import numpy as np
import ml_dtypes

import concourse.bacc as bacc
import concourse.bass as bass
import concourse.mybir as mybir
import concourse.tile as tile
from concourse.bass_utils import run_bass_kernel_spmd

F32 = mybir.dt.float32
BF = mybir.dt.bfloat16

B, L, D, H, DK = 4, 2048, 1024, 16, 64
HD = 8            # heads per core
GW = 512          # head-group width (8 heads * 64)
NCH = D // 128    # 8 contraction chunks
QS = 512          # q-slice width in attention
NQS = L // QS     # 4
NKB = L // 128    # 16 key blocks
NTT = L // 128    # 16 token tiles


def _build_nc(dbg=False, phases="ABC"):
    nc = bacc.Bacc("TRN2", target_bir_lowering=False, debug=False, num_devices=8)

    xq = nc.dram_tensor("xq", [D, L], BF, kind="ExternalInput").ap()
    xk = nc.dram_tensor("xk", [D, L], BF, kind="ExternalInput").ap()
    xv = nc.dram_tensor("xv", [D, L], BF, kind="ExternalInput").ap()
    wq = nc.dram_tensor("wq", [D, GW], BF, kind="ExternalInput").ap()
    wk = nc.dram_tensor("wk", [D, GW], BF, kind="ExternalInput").ap()
    wv = nc.dram_tensor("wv", [D, GW], BF, kind="ExternalInput").ap()
    wo = nc.dram_tensor("wo", [GW, D], BF, kind="ExternalInput").ap()
    bq = nc.dram_tensor("bq", [128, 4], F32, kind="ExternalInput").ap()
    bk = nc.dram_tensor("bk", [128, 4], F32, kind="ExternalInput").ap()
    vb = nc.dram_tensor("vb", [GW], F32, kind="ExternalInput").ap()
    msk = nc.dram_tensor("msk", [128, 128], BF, kind="ExternalInput").ap()
    idm = nc.dram_tensor("idm", [128, 128], BF, kind="ExternalInput").ap()
    outp = nc.dram_tensor("outp", [L, D], BF, kind="ExternalOutput").ap()
    if dbg:
        qt_dbg = nc.dram_tensor("qt_dbg", [128, 4 * L], BF, kind="ExternalOutput").ap()
        kt_dbg = nc.dram_tensor("kt_dbg", [128, 4 * L], BF, kind="ExternalOutput").ap()
        v2_dbg = nc.dram_tensor("v2_dbg", [128, NKB * HD * 128], BF,
                                kind="ExternalOutput").ap()
        ctx_dbg = nc.dram_tensor("ctx_dbg", [128, 4 * L], BF, kind="ExternalOutput").ap()

    with tile.TileContext(nc) as tc:
        with (
            tc.tile_pool(name="persist", bufs=1) as persist,
            tc.tile_pool(name="xin", bufs=3) as xinp,
            tc.tile_pool(name="pt", bufs=5) as ptp,
            tc.tile_pool(name="outs", bufs=6) as outsp,
            tc.tile_pool(name="psQK", bufs=2, space="PSUM") as psQK,
            tc.tile_pool(name="cb", bufs=4) as cbp,
            tc.tile_pool(name="rc", bufs=2) as rcp,
            tc.tile_pool(name="psS", bufs=2, space="PSUM") as psS,
            tc.tile_pool(name="psAV", bufs=2, space="PSUM") as psAV,
        ):
            # ---- persistent SBUF ----
            wq_s = persist.tile([128, NCH, GW], BF, tag="wq")
            wk_s = persist.tile([128, NCH, GW], BF, tag="wk")
            wv_s = persist.tile([128, NCH, GW], BF, tag="wv")
            wo_s = persist.tile([128, 4, D], BF, tag="wo")
            qt_s = persist.tile([128, 4, L], BF, tag="qt")
            kt_s = persist.tile([128, 4, L], BF, tag="kt")
            ctx_s = persist.tile([128, 4, L], BF, tag="ctx")
            v2_s = persist.tile([128, NKB, HD, 128], BF, tag="v2")
            bq_s = persist.tile([128, 4], F32, tag="bq")
            bk_s = persist.tile([128, 4], F32, tag="bk")
            vb_s = persist.tile([128, 2, 4, DK], F32, tag="vb")
            msk_s = persist.tile([128, 128], BF, tag="msk")
            idm_s = persist.tile([128, 128], BF, tag="idm")

            # ---------- phase A emission helpers ----------
            def stage_A_dmas(n, split=False):
                """Issue the x DMAs for token-slice n; return the SBUF tiles."""
                xt = {}
                for nm, src in (("q", xq), ("k", xk), ("v", xv)):
                    t = xinp.tile([128, NCH, QS], BF, tag=f"x{nm}",
                                  name=f"x{nm}{n}")
                    halves = ((0, 4), (4, 8)) if split else ((0, 8),)
                    for c0, c1 in halves:
                        nc.sync.dma_start(
                            t[:, c0:c1, :],
                            src[c0 * 128:c1 * 128,
                                n * QS:(n + 1) * QS].rearrange(
                                "(c p) t -> p c t", p=128))
                    xt[nm] = t
                return xt

            # startup: interleave first-slice x loads with their weights in
            # chunk-halves so the first matmuls start after ~2 half loads
            def _wload(dst, src, c0, c1):
                nc.sync.dma_start(
                    dst[:, c0:c1, :],
                    src[c0 * 128:c1 * 128, :].rearrange("(c p) n -> p c n", p=128))

            vb_bcast = bass.AP(tensor=vb.tensor, offset=vb.offset,
                               ap=[[0, 128], [256, 2], [DK, 4], [1, DK]])
            nc.gpsimd.dma_start(vb_s[:, :, :, :], vb_bcast)
            # ones-columns of v2 (softmax denominator trick), preset once
            nc.gpsimd.memset(v2_s[:, :, :, DK:128], 1.0)

            xt0 = {}
            first = True
            for nm, xsrc, wdst, wsrc in (("q", xq, wq_s, wq),
                                         ("k", xk, wk_s, wk),
                                         ("v", xv, wv_s, wv)):
                t = xinp.tile([128, NCH, QS], BF, tag=f"x{nm}", name=f"x{nm}0")
                for c0, c1 in ((0, 4), (4, 8)):
                    _wload(wdst, wsrc, c0, c1)
                    nc.sync.dma_start(
                        t[:, c0:c1, :],
                        xsrc[c0 * 128:c1 * 128, 0:QS].rearrange(
                            "(c p) t -> p c t", p=128))
                    if first:
                        # tiny constant loads right after the first half-load
                        # pair — bias-adds gate PSUM slot recycling, but the
                        # very first matmul only needs wq/xq chunks 0:4
                        nc.sync.dma_start(bq_s[:, :], bq[:, :])
                        nc.sync.dma_start(bk_s[:, :], bk[:, :])
                        nc.sync.dma_start(msk_s[:, :], msk[:, :])
                        nc.sync.dma_start(idm_s[:, :], idm[:, :])
                        first = False
                xt0[nm] = t

            def stage_A_units(n, xt, qk_only=False):
                """Compute units (closures) for token-slice n."""
                units = []
                for (nm, w_s, dst, b_s) in (("q", wq_s, qt_s, bq_s),
                                            ("k", wk_s, kt_s, bk_s)):
                    xti = xt[nm]
                    mms, fins = [], []
                    for m in range(4):
                        ps = psQK.tile([128, QS], F32, tag="ps",
                                       name=f"psA_{nm}{n}_{m}")

                        def mm(c0, nm=nm, m=m, ps=ps, xti=xti, w_s=w_s):
                            for c in range(c0, c0 + 4):
                                nc.tensor.matmul(
                                    ps[:, :],
                                    w_s[:, c, m * 128:(m + 1) * 128],
                                    xti[:, c, :],
                                    start=(c == 0), stop=(c == NCH - 1))

                        def fin(nm=nm, m=m, n=n, ps=ps, dst=dst, b_s=b_s):
                            nc.vector.tensor_scalar_add(
                                dst[:, m, n * QS:(n + 1) * QS],
                                ps[:, :], b_s[:, m:m + 1])

                        mms.append(mm)
                        fins.append(fin)
                    # pair m-tiles: both first-halves, then both second-halves
                    # (+bias) — lets compute start when only chunks 0:4 of the
                    # weights/inputs have landed (startup half-loads)
                    for m0 in (0, 2):
                        units.append((lambda mm=mms[m0]: mm(0), 853))
                        units.append((lambda mm=mms[m0 + 1]: mm(0), 853))
                        units.append((lambda mm=mms[m0], fin=fins[m0]:
                                      (mm(4), fin()), 853))
                        units.append((lambda mm=mms[m0 + 1], fin=fins[m0 + 1]:
                                      (mm(4), fin()), 853))
                if qk_only:
                    return units
                units.extend(stage_V_units(n, xt))
                return units

            def stage_V_units(n, xt):
                units = []
                # V: per token-tile tt (128 tokens), half hf (4 heads)
                xtv = xt["v"]
                for tt in range(4):
                    tg = n * 4 + tt
                    for hf in range(2):
                        ps = psQK.tile([128, QS], F32, tag="ps",
                                       name=f"psV{tg}_{hf}")

                        def mmv(c0, tt=tt, hf=hf, ps=ps, xtv=xtv):
                            for c in range(c0, c0 + 4):
                                nc.tensor.matmul(
                                    ps[:, 0:256],
                                    xtv[:, c, tt * 128:(tt + 1) * 128],
                                    wv_s[:, c, hf * 256:(hf + 1) * 256],
                                    start=(c == 0), stop=(c == NCH - 1))

                        def finv(tg=tg, hf=hf, ps=ps):
                            nc.vector.tensor_add(
                                v2_s[:, tg, hf * 4:(hf + 1) * 4, 0:DK],
                                ps[:, 0:256], vb_s[:, hf, :, :])

                        units.append((lambda mmv=mmv: mmv(0), 427))
                        units.append((lambda mmv=mmv, finv=finv:
                                      (mmv(4), finv()), 427))
                return units

            # ---------- phase C emission helpers ----------
            def stage_C_units(t, tail=False):
                units = []
                # in the tail, B is done: borrow the score pool (3 bufs) so
                # psum slot recycling never gates the matmuls
                pool, tg = (psS, "sp") if tail else (psQK, "ps")
                for n2 in range(2):
                    ps = pool.tile([128, QS], F32, tag=tg,
                                   name=f"psC{t}_{n2}")

                    def mmc(ps=ps, t=t, n2=n2):
                        for c in range(4):
                            nc.tensor.matmul(
                                ps[:, :],
                                ctx_s[:, c, t * 128:(t + 1) * 128],
                                wo_s[:, c, n2 * QS:(n2 + 1) * QS],
                                start=(c == 0), stop=(c == 3))

                    def finc(ps=ps, t=t, n2=n2, tail=tail):
                        ot = outsp.tile([128, QS], BF, tag="ot",
                                        name=f"ot{t}_{n2}")
                        # scalar engine is idle in the tail; DVE during B.
                        # quarter-split the very last tile to shorten the
                        # copy->DMA drain after the final matmul
                        nq = 1
                        w = QS // nq
                        for i in range(nq):
                            if tail:
                                nc.scalar.copy(ot[:, i * w:(i + 1) * w],
                                               ps[:, i * w:(i + 1) * w])
                            else:
                                nc.vector.tensor_copy(ot[:, i * w:(i + 1) * w],
                                                      ps[:, i * w:(i + 1) * w])
                            nc.sync.dma_start(
                                outp[t * 128:(t + 1) * 128,
                                     n2 * QS + i * w:n2 * QS + (i + 1) * w],
                                ot[:, i * w:(i + 1) * w])

                    units.append((mmc, 853))
                    units.append((finc, 0))
                return units

            # ---------- filler pump ----------
            filler = []
            pump_acc = [0.0]

            def pump(budget_ns):
                pump_acc[0] += budget_ns
                while filler and pump_acc[0] >= filler[0][1]:
                    fn, w = filler.pop(0)
                    fn()
                    pump_acc[0] -= w
                if not filler:
                    pump_acc[0] = 0.0

            # ---------- phase B ----------
            # Global score->exp->AV software pipeline per q-slice.  Off-
            # diagonal score blocks are PAIRED into one 2-bank psum tile so a
            # single exp covers 1024 columns (halves ACT per-instruction
            # overhead).  AV accumulates [128, q]: rows 0:64 ctx, rows 64:128
            # the softmax denominator replicated via the 64 ones-columns of
            # v2_s -> normalize is one tensor-tensor divide, no broadcast.
            def emit_B_qs(qs, rate):
                nkb = 4 * qs + 4
                st_units, av_units = [], []
                for h in range(HD):
                    po = (h % 2) * 64
                    mc = h // 2
                    cpsd = {}
                    pts = [None] * nkb

                    def score_mm(sp2, j, kb, col0, po=po, mc=mc):
                        nc.tensor.matmul(
                            sp2[:, j, col0:],
                            kt_s[po:po + 64, mc, kb * 128:(kb + 1) * 128],
                            qt_s[po:po + 64, mc, qs * QS + col0:(qs + 1) * QS],
                            start=True, stop=True)

                    def emit_pair(kb, h=h, pts=pts, smm=score_mm):
                        sp2 = psS.tile([128, 2, QS], F32, tag="sp",
                                       name=f"sp{h}_{qs}_{kb}")
                        smm(sp2, 0, kb, 0)
                        smm(sp2, 1, kb + 1, 0)
                        pt2 = ptp.tile([128, 2, QS], BF, tag="pt",
                                       name=f"pt{h}_{qs}_{kb}")
                        nc.scalar.activation(
                            pt2[:, :, :], sp2[:, :, :],
                            func=mybir.ActivationFunctionType.Exp)
                        pts[kb] = (pt2, 0)
                        pts[kb + 1] = (pt2, 1)

                    def emit_single(kb, h=h, pts=pts, smm=score_mm):
                        col0 = max(0, kb * 128 - qs * QS)
                        sp2 = psS.tile([128, 2, QS], F32, tag="sp",
                                       name=f"sp{h}_{qs}_{kb}")
                        smm(sp2, 0, kb, col0)
                        pt2 = ptp.tile([128, 2, QS], BF, tag="pt",
                                       name=f"pt{h}_{qs}_{kb}")
                        nc.scalar.activation(
                            pt2[:, 0, col0:], sp2[:, 0, col0:],
                            func=mybir.ActivationFunctionType.Exp)
                        nc.vector.tensor_mul(
                            pt2[:, 0, col0:col0 + 128],
                            pt2[:, 0, col0:col0 + 128], msk_s[:, :])
                        pts[kb] = (pt2, 0)

                    def emit_av(kb, h=h, po=po, mc=mc, pts=pts, cpsd=cpsd):
                        if kb == 0:
                            # allocate at emission time so the pool sees the
                            # slot-reuse hazard against the previous head
                            cpsd["t"] = psAV.tile([128, QS], F32, tag="cps",
                                                  name=f"cps{h}_{qs}")
                        cps = cpsd["t"]
                        col0 = max(0, kb * 128 - qs * QS)
                        pt2, j = pts[kb]
                        last = kb == nkb - 1
                        if col0 > 0 or kb == 4 * qs:
                            # diagonal block: the masked 128-wide window goes
                            # LAST so the unmasked part starts right after exp
                            if col0 + 128 < QS:
                                nc.tensor.matmul(
                                    cps[:, col0 + 128:],
                                    v2_s[:, kb, h, :],
                                    pt2[:, j, col0 + 128:],
                                    start=(kb == 0), stop=False)
                            nc.tensor.matmul(
                                cps[:, col0:col0 + 128],
                                v2_s[:, kb, h, :],
                                pt2[:, j, col0:col0 + 128],
                                start=False, stop=last)
                        else:
                            nc.tensor.matmul(
                                cps[:, col0:],
                                v2_s[:, kb, h, :],
                                pt2[:, j, col0:],
                                start=(kb == 0), stop=last)
                        pts[kb] = None
                        if last:
                            # rows 0:64 ctx, rows 64:128 replicated
                            # denominator.  DVE may read only ONE non-scalar
                            # PSUM input per instruction -> recip to SBUF,
                            # then psum*sbuf multiply.
                            rc = rcp.tile([64, QS], F32, tag="rc",
                                          name=f"rc{h}_{qs}")
                            nc.vector.reciprocal(rc[:, :], cps[64:128, :])
                            nc.vector.tensor_mul(
                                ctx_s[po:po + 64, mc, qs * QS:(qs + 1) * QS],
                                cps[0:64, :], rc[:, :])

                    for kb in range(0, 4 * qs, 2):
                        st_units.append((lambda kb=kb, f=emit_pair: f(kb), 2))
                    for kb in range(4 * qs, nkb):
                        st_units.append((lambda kb=kb, f=emit_single: f(kb), 1))
                    for kb in range(nkb):
                        av_units.append(lambda kb=kb, f=emit_av: f(kb))

                LEAD = 4
                scored, avd, total = 0, 0, len(av_units)
                while avd < total:
                    if st_units and scored - avd < LEAD:
                        f, k = st_units.pop(0)
                        f()
                        scored += k
                    else:
                        av_units.pop(0)()
                        avd += 1
                    pump(rate)

            # ---------- main schedule ----------
            xt1 = stage_A_dmas(1)
            nc.sync.dma_start(wo_s[:, :, :],
                              wo.rearrange("(c p) n -> p c n", p=128))
            for u, _w in stage_A_units(0, xt0):
                u()
            xts = {1: xt1}

            if "B" in phases:
                for qs in range(NQS):
                    # prefetch x for slice qs+2
                    if qs + 2 < NQS:
                        xts[qs + 2] = stage_A_dmas(qs + 2)
                    # filler: A(qs+1) compute, then C tiles once A is done
                    if qs + 1 < NQS:
                        filler.extend(stage_A_units(qs + 1, xts.pop(qs + 1)))
                    elif "C" in phases:
                        for t in range(12):
                            filler.extend(stage_C_units(t))
                    # driver iterations: st units (pairs+singles) + av units
                    nsteps = 8 * ((2 * qs + 4) + (4 * qs + 4))
                    rate = sum(w for _, w in filler) / nsteps + 1e-9
                    emit_B_qs(qs, rate)
                    while filler:
                        filler.pop(0)[0]()

            if "C" in phases:
                t0 = 12 if "B" in phases else 0
                for t in range(t0, NTT):
                    for u, _w in stage_C_units(t, tail=True):
                        u()

            if dbg:
                nc.sync.dma_start(qt_dbg[:, :], qt_s[:, :, :])
                nc.sync.dma_start(kt_dbg[:, :], kt_s[:, :, :])
                nc.sync.dma_start(v2_dbg[:, :], v2_s[:, :, :, :])
                nc.sync.dma_start(ctx_dbg[:, :], ctx_s[:, :, :])

    nc.compile()
    return nc


_NC = None
LAST_RESULTS = None


def _host_maps(inputs):
    bf = ml_dtypes.bfloat16
    f = lambda a: np.asarray(a, dtype=np.float32)
    q, k, v = f(inputs["q"]), f(inputs["k"]), f(inputs["v"])
    wq_w, wq_b = f(inputs["wq_w"]), f(inputs["wq_b"])
    wk_w, wk_b = f(inputs["wk_w"]), f(inputs["wk_b"])
    wv_w, wv_b = f(inputs["wv_w"]), f(inputs["wv_b"])
    wo_w = f(inputs["wo_w"])

    msk = np.ascontiguousarray(
        (np.arange(128)[None, :] >= np.arange(128)[:, None])).astype(bf)

    gmaps = []
    for g in range(2):
        sl = slice(g * GW, (g + 1) * GW)
        gmaps.append(dict(
            wq=np.ascontiguousarray((wq_w[sl] * 0.125).T).astype(bf),
            wk=np.ascontiguousarray(wk_w[sl].T).astype(bf),
            wv=np.ascontiguousarray(wv_w[sl].T).astype(bf),
            wo=np.ascontiguousarray(wo_w[:, sl].T).astype(bf),
            bq=np.ascontiguousarray((wq_b[sl] * 0.125).reshape(4, 128).T),
            bk=np.ascontiguousarray(wk_b[sl].reshape(4, 128).T),
            vb=np.ascontiguousarray(wv_b[sl]),
            msk=msk, idm=np.eye(128, dtype=np.float32).astype(bf)))

    bmaps = []
    for b in range(B):
        bmaps.append(dict(
            xq=np.ascontiguousarray(q[b].T).astype(bf),
            xk=np.ascontiguousarray(k[b].T).astype(bf),
            xv=np.ascontiguousarray(v[b].T).astype(bf)))

    return [dict(**bmaps[c // 2], **gmaps[c % 2]) for c in range(8)]


def kernel(**inputs):
    global _NC, LAST_RESULTS
    import os
    if _NC is None:
        _NC = _build_nc()

    in_maps = _host_maps(inputs)
    trace = bool(int(os.environ.get("KERNEL_TRACE", "0")))
    res = run_bass_kernel_spmd(_NC, in_maps, list(range(8)), trace=trace)
    LAST_RESULTS = res

    wo_b = np.asarray(inputs["wo_b"], dtype=np.float32)
    out = np.empty((B, L, D), np.float32)
    for b in range(B):
        out[b] = (np.asarray(res.results[2 * b]["outp"], np.float32)
                  + np.asarray(res.results[2 * b + 1]["outp"], np.float32)
                  + wo_b[None, :])
    return out


# revision 4
# speedup vs baseline: 1.7526x; 1.0148x over previous
"""Causal MHA (B=4, L=2048, D=1024, H=16) on 8 NeuronCores — v2, all bf16.

Sharding: core c -> (batch b = c//2, head-group g = c%2); 8 heads per core.
Host sums the two head-group partial outputs per batch and adds wo_b.

Per-core structure (single Bass module, software-pipelined emission):
  A(n): per 512-token slice n: QT/KT = w @ x chunks -> psum -> +bias -> SBUF
        bf16 [128, 4, L] (head-dim on partitions).  V2 = x.T @ wv -> psum
        [tok,4,64] -> +bias -> SBUF v2[128 tok, kb, 8 heads, 128] where cols
        64:128 of each head slot are constant 1.0 (preset once): the AV
        matmul then yields the softmax denominator replicated on 64
        partitions for free.
  B(h, qs): score S.T[keys,q] = KT_h.T @ QT_h (causal-trimmed), exp on
        ACT -> pt bf16, tri-mask on diagonal blocks (DVE), AV psum
        [128, q] += v2[kb,h].T @ pt: rows 0:64 ctx, 64:128 denominator.
        ctx = rows/denominator (one DVE divide) -> SBUF ctx bf16.
  C(t): out[tok128, 1024] = sum_c ctx[c].T @ wo[c] -> psum -> bf16 -> DRAM.
  A(n+1) and C(t) matmuls are interleaved into B's emission as filler so
  the PE never waits on the exp (ACT) pipeline.
"""

import numpy as np
import ml_dtypes

import concourse.bacc as bacc
import concourse.bass as bass
import concourse.mybir as mybir
import concourse.tile as tile
from concourse.bass_utils import run_bass_kernel_spmd

F32 = mybir.dt.float32
BF = mybir.dt.bfloat16

B, L, D, H, DK = 4, 2048, 1024, 16, 64
HD = 8            # heads per core
GW = 512          # head-group width (8 heads * 64)
NCH = D // 128    # 8 contraction chunks
QS = 512          # q-slice width in attention
NQS = L // QS     # 4
NKB = L // 128    # 16 key blocks
NTT = L // 128    # 16 token tiles


def _build_nc(dbg=False, phases="ABC"):
    nc = bacc.Bacc("TRN2", target_bir_lowering=False, debug=False, num_devices=8)

    xq = nc.dram_tensor("xq", [D, L], BF, kind="ExternalInput").ap()
    xk = nc.dram_tensor("xk", [D, L], BF, kind="ExternalInput").ap()
    xv = nc.dram_tensor("xv", [D, L], BF, kind="ExternalInput").ap()
    wq = nc.dram_tensor("wq", [D, GW], BF, kind="ExternalInput").ap()
    wk = nc.dram_tensor("wk", [D, GW], BF, kind="ExternalInput").ap()
    wv = nc.dram_tensor("wv", [D, GW], BF, kind="ExternalInput").ap()
    wo = nc.dram_tensor("wo", [GW, D], BF, kind="ExternalInput").ap()
    bq = nc.dram_tensor("bq", [128, 4], F32, kind="ExternalInput").ap()
    bk = nc.dram_tensor("bk", [128, 4], F32, kind="ExternalInput").ap()
    vb = nc.dram_tensor("vb", [GW], F32, kind="ExternalInput").ap()
    msk = nc.dram_tensor("msk", [128, 128], BF, kind="ExternalInput").ap()
    idm = nc.dram_tensor("idm", [128, 128], BF, kind="ExternalInput").ap()
    outp = nc.dram_tensor("outp", [L, D], BF, kind="ExternalOutput").ap()
    if dbg:
        qt_dbg = nc.dram_tensor("qt_dbg", [128, 4 * L], BF, kind="ExternalOutput").ap()
        kt_dbg = nc.dram_tensor("kt_dbg", [128, 4 * L], BF, kind="ExternalOutput").ap()
        v2_dbg = nc.dram_tensor("v2_dbg", [128, NKB * HD * 128], BF,
                                kind="ExternalOutput").ap()
        ctx_dbg = nc.dram_tensor("ctx_dbg", [128, 4 * L], BF, kind="ExternalOutput").ap()

    with tile.TileContext(nc) as tc:
        with (
            tc.tile_pool(name="persist", bufs=1) as persist,
            tc.tile_pool(name="xin", bufs=3) as xinp,
            tc.tile_pool(name="pt", bufs=8) as ptp,
            tc.tile_pool(name="outs", bufs=6) as outsp,
            tc.tile_pool(name="psQK", bufs=2, space="PSUM") as psQK,
            tc.tile_pool(name="cb", bufs=4) as cbp,
            tc.tile_pool(name="rc", bufs=2) as rcp,
            tc.tile_pool(name="psS", bufs=3, space="PSUM") as psS,
            tc.tile_pool(name="psT", bufs=1, space="PSUM") as psT,
            tc.tile_pool(name="psAV", bufs=2, space="PSUM") as psAV,
        ):
            # ---- persistent SBUF ----
            wq_s = persist.tile([128, NCH, GW], BF, tag="wq")
            wk_s = persist.tile([128, NCH, GW], BF, tag="wk")
            wv_s = persist.tile([128, NCH, GW], BF, tag="wv")
            wo_s = persist.tile([128, 4, D], BF, tag="wo")
            qt_s = persist.tile([128, 4, L], BF, tag="qt")
            kt_s = persist.tile([128, 4, L], BF, tag="kt")
            ctx_s = persist.tile([128, 4, L], BF, tag="ctx")
            v2_s = persist.tile([128, NKB, HD, 128], BF, tag="v2")
            bq_s = persist.tile([128, 4], F32, tag="bq")
            bk_s = persist.tile([128, 4], F32, tag="bk")
            vb_s = persist.tile([128, 2, 4, DK], F32, tag="vb")
            msk_s = persist.tile([128, 128], BF, tag="msk")
            idm_s = persist.tile([128, 128], BF, tag="idm")

            # ---------- phase A emission helpers ----------
            def stage_A_dmas(n, split=False):
                """Issue the x DMAs for token-slice n; return the SBUF tiles."""
                xt = {}
                for nm, src in (("q", xq), ("k", xk), ("v", xv)):
                    t = xinp.tile([128, NCH, QS], BF, tag=f"x{nm}",
                                  name=f"x{nm}{n}")
                    halves = ((0, 4), (4, 8)) if split else ((0, 8),)
                    for c0, c1 in halves:
                        nc.sync.dma_start(
                            t[:, c0:c1, :],
                            src[c0 * 128:c1 * 128,
                                n * QS:(n + 1) * QS].rearrange(
                                "(c p) t -> p c t", p=128))
                    xt[nm] = t
                return xt

            # startup: interleave first-slice x loads with their weights in
            # chunk-halves so the first matmuls start after ~2 half loads
            def _wload(dst, src, c0, c1):
                nc.sync.dma_start(
                    dst[:, c0:c1, :],
                    src[c0 * 128:c1 * 128, :].rearrange("(c p) n -> p c n", p=128))

            vb_bcast = bass.AP(tensor=vb.tensor, offset=vb.offset,
                               ap=[[0, 128], [256, 2], [DK, 4], [1, DK]])
            nc.gpsimd.dma_start(vb_s[:, :, :, :], vb_bcast)
            # ones-columns of v2 (softmax denominator trick), preset once
            nc.gpsimd.memset(v2_s[:, :, :, DK:128], 1.0)

            xt0 = {}
            first = True
            for nm, xsrc, wdst, wsrc in (("q", xq, wq_s, wq),
                                         ("k", xk, wk_s, wk),
                                         ("v", xv, wv_s, wv)):
                t = xinp.tile([128, NCH, QS], BF, tag=f"x{nm}", name=f"x{nm}0")
                quarters = ((0, 4), (4, 8))
                for c0, c1 in quarters:
                    _wload(wdst, wsrc, c0, c1)
                    nc.sync.dma_start(
                        t[:, c0:c1, :],
                        xsrc[c0 * 128:c1 * 128, 0:QS].rearrange(
                            "(c p) t -> p c t", p=128))
                    if first:
                        # tiny constant loads right after the first quarter —
                        # bias-adds gate PSUM slot recycling, but the very
                        # first matmuls only need wq/xq chunks 0:2
                        nc.sync.dma_start(bq_s[:, :], bq[:, :])
                        nc.sync.dma_start(bk_s[:, :], bk[:, :])
                        nc.sync.dma_start(msk_s[:, :], msk[:, :])
                        nc.sync.dma_start(idm_s[:, :], idm[:, :])
                        first = False
                xt0[nm] = t

            def stage_A_units(n, xt, qk_only=False):
                """Compute units (closures) for token-slice n."""
                units = []
                for (nm, w_s, dst, b_s) in (("q", wq_s, qt_s, bq_s),
                                            ("k", wk_s, kt_s, bk_s)):
                    xti = xt[nm]
                    mms, fins = [], []
                    for m in range(4):
                        ps = psQK.tile([128, QS], F32, tag="ps",
                                       name=f"psA_{nm}{n}_{m}")

                        def mm(c0, c1, nm=nm, m=m, ps=ps, xti=xti, w_s=w_s):
                            for c in range(c0, c1):
                                nc.tensor.matmul(
                                    ps[:, :],
                                    w_s[:, c, m * 128:(m + 1) * 128],
                                    xti[:, c, :],
                                    start=(c == 0), stop=(c == NCH - 1))

                        def fin(nm=nm, m=m, n=n, ps=ps, dst=dst, b_s=b_s):
                            nc.vector.tensor_scalar_add(
                                dst[:, m, n * QS:(n + 1) * QS],
                                ps[:, :], b_s[:, m:m + 1])

                        mms.append(mm)
                        fins.append(fin)
                    # pair m-tiles: both first-halves, then both second-halves
                    # (+bias) — lets compute start when only the first chunks
                    # of the weights/inputs have landed (startup split loads)
                    if True:
                        for m0 in (0, 2):
                            units.append((lambda mm=mms[m0]: mm(0, 4), 853))
                            units.append((lambda mm=mms[m0 + 1]: mm(0, 4), 853))
                            units.append((lambda mm=mms[m0], fin=fins[m0]:
                                          (mm(4, 8), fin()), 853))
                            units.append((lambda mm=mms[m0 + 1],
                                          fin=fins[m0 + 1]:
                                          (mm(4, 8), fin()), 853))
                if qk_only:
                    return units
                units.extend(stage_V_units(n, xt))
                return units

            def stage_V_units(n, xt):
                units = []
                # V: per token-tile tt (128 tokens), half hf (4 heads)
                xtv = xt["v"]
                for tt in range(4):
                    tg = n * 4 + tt
                    for hf in range(2):
                        ps = psQK.tile([128, QS], F32, tag="ps",
                                       name=f"psV{tg}_{hf}")

                        def mmv(c0, tt=tt, hf=hf, ps=ps, xtv=xtv):
                            for c in range(c0, c0 + 4):
                                nc.tensor.matmul(
                                    ps[:, 0:256],
                                    xtv[:, c, tt * 128:(tt + 1) * 128],
                                    wv_s[:, c, hf * 256:(hf + 1) * 256],
                                    start=(c == 0), stop=(c == NCH - 1))

                        def finv(tg=tg, hf=hf, ps=ps):
                            nc.vector.tensor_add(
                                v2_s[:, tg, hf * 4:(hf + 1) * 4, 0:DK],
                                ps[:, 0:256], vb_s[:, hf, :, :])

                        units.append((lambda mmv=mmv: mmv(0), 427))
                        units.append((lambda mmv=mmv, finv=finv:
                                      (mmv(4), finv()), 427))
                return units

            # ---------- phase C emission helpers ----------
            def stage_C_units(t, tail=False):
                units = []
                # in the tail, B is done: borrow the score pool (3 bufs) so
                # psum slot recycling never gates the matmuls
                pool, tg = (psS, "sp") if tail else (psQK, "ps")
                for n2 in range(2):
                    ps = pool.tile([128, QS], F32, tag=tg,
                                   name=f"psC{t}_{n2}")

                    def mmc(ps=ps, t=t, n2=n2):
                        for c in range(4):
                            nc.tensor.matmul(
                                ps[:, :],
                                ctx_s[:, c, t * 128:(t + 1) * 128],
                                wo_s[:, c, n2 * QS:(n2 + 1) * QS],
                                start=(c == 0), stop=(c == 3))

                    def finc(ps=ps, t=t, n2=n2, tail=tail):
                        ot = outsp.tile([128, QS], BF, tag="ot",
                                        name=f"ot{t}_{n2}")
                        # scalar engine is idle in the tail; DVE during B.
                        # quarter-split the very last tile to shorten the
                        # copy->DMA drain after the final matmul
                        nq = 1
                        w = QS // nq
                        for i in range(nq):
                            if tail:
                                nc.scalar.copy(ot[:, i * w:(i + 1) * w],
                                               ps[:, i * w:(i + 1) * w])
                            else:
                                nc.vector.tensor_copy(ot[:, i * w:(i + 1) * w],
                                                      ps[:, i * w:(i + 1) * w])
                            nc.sync.dma_start(
                                outp[t * 128:(t + 1) * 128,
                                     n2 * QS + i * w:n2 * QS + (i + 1) * w],
                                ot[:, i * w:(i + 1) * w])

                    units.append((mmc, 853))
                    units.append((finc, 0))
                return units

            # ---------- filler pump ----------
            filler = []
            pump_acc = [0.0]

            def pump(budget_ns):
                pump_acc[0] += budget_ns
                while filler and pump_acc[0] >= filler[0][1]:
                    fn, w = filler.pop(0)
                    fn()
                    pump_acc[0] -= w
                if not filler:
                    pump_acc[0] = 0.0

            # ---------- phase B ----------
            # Global score->exp->AV software pipeline per q-slice, hybrid AV:
            #  - qs=0: classic orientation (moving dim = q) — the V/A(1)
            #    filler makes qs0 PE-paced, heavy AVs act as their own filler.
            #  - qs>=1: flipped orientation cps2[q-part, qt*65:(qt+1)*65] +=
            #    P.T @ V (moving dim 65): ~56k fewer PE cycles; column 64 is
            #    the softmax denominator (ones-column of v2_s); per-partition
            #    tensor-scalar divide + PE transpose restore [hd, tok].
            # PSUM pending-zero discipline: exactly ONE start=True (first
            # write of kb=0 — it touches every region of the bank) and ONE
            # stop=True (last write) per cps accumulation bank.
            def emit_B_qs(qs, rate):
                nkb = 4 * qs + 4
                st_units, av_units = [], []
                for h in range(HD):
                    po = (h % 2) * 64
                    mc = h // 2
                    cpsd = {}
                    pts = [None] * nkb

                    def score_mm(sp, kb, col0, po=po, mc=mc):
                        nc.tensor.matmul(
                            sp[:, col0:],
                            kt_s[po:po + 64, mc, kb * 128:(kb + 1) * 128],
                            qt_s[po:po + 64, mc, qs * QS + col0:(qs + 1) * QS],
                            start=True, stop=True)

                    def emit_st(kb, h=h, pts=pts, smm=score_mm):
                        col0 = max(0, kb * 128 - qs * QS)
                        sp = psS.tile([128, QS], F32, tag="sp",
                                      name=f"sp{h}_{qs}_{kb}")
                        smm(sp, kb, col0)
                        pt = ptp.tile([128, QS], BF, tag="pt",
                                      name=f"pt{h}_{qs}_{kb}")
                        nc.scalar.activation(
                            pt[:, col0:], sp[:, col0:],
                            func=mybir.ActivationFunctionType.Exp)
                        if col0 > 0 or kb == 4 * qs:
                            nc.vector.tensor_mul(
                                pt[:, col0:col0 + 128],
                                pt[:, col0:col0 + 128], msk_s[:, :])
                        pts[kb] = pt

                    def emit_av_b(kb, h=h, po=po, mc=mc, pts=pts, cpsd=cpsd):
                        if kb == 0:
                            cpsd["t"] = psAV.tile([128, QS], F32, tag="cps",
                                                  name=f"cps{h}_{qs}")
                        cps2 = cpsd["t"]
                        qt0 = max(0, kb - 4 * qs)
                        pt = pts[kb]
                        # masked qt0 tile last so unmasked AVs follow exp
                        order = list(range(qt0 + 1, 4)) + [qt0]
                        for i, qt in enumerate(order):
                            nc.tensor.matmul(
                                cps2[:, qt * 65:(qt + 1) * 65],
                                pt[:, qt * 128:(qt + 1) * 128],
                                v2_s[:, kb, h, 0:DK + 1],
                                start=(kb == 0 and i == 0),
                                stop=(kb == nkb - 1 and qt == qt0))
                        pts[kb] = None
                        if kb == nkb - 1:
                            # bank-wide stop just fired: normalize all four
                            # q-tiles, transpose each to [hd, tok], store
                            for qt in range(4):
                                ctxB = cbp.tile([128, DK], BF, tag="cb",
                                                name=f"cb{h}_{qs}_{qt}")
                                rc2 = rcp.tile([128, 4], F32, tag="rcb",
                                               name=f"rcb{h}_{qs}_{qt}")
                                nc.vector.reciprocal(
                                    rc2[:, 0:1],
                                    cps2[:, qt * 65 + DK:qt * 65 + DK + 1])
                                nc.vector.tensor_scalar_mul(
                                    ctxB[:, :], cps2[:, qt * 65:qt * 65 + DK],
                                    rc2[:, 0:1])
                                tr = psT.tile([DK, 128], BF, tag="tr",
                                              name=f"tr{h}_{qs}_{qt}")
                                nc.tensor.matmul(tr[:, :], ctxB[:, :],
                                                 idm_s[:, :],
                                                 is_transpose=True)
                                nc.vector.tensor_copy(
                                    ctx_s[po:po + 64, mc,
                                          qs * QS + qt * 128:
                                          qs * QS + (qt + 1) * 128],
                                    tr[:, :])

                    def emit_av_a(kb, h=h, po=po, mc=mc, pts=pts, cpsd=cpsd):
                        if kb == 0:
                            cpsd["t"] = psAV.tile([128, QS], F32, tag="cps",
                                                  name=f"cps{h}_{qs}")
                        cps = cpsd["t"]
                        col0 = max(0, kb * 128 - qs * QS)
                        pt = pts[kb]
                        last = kb == nkb - 1
                        if col0 > 0 or kb == 4 * qs:
                            if col0 + 128 < QS:
                                nc.tensor.matmul(
                                    cps[:, col0 + 128:],
                                    v2_s[:, kb, h, :],
                                    pt[:, col0 + 128:],
                                    start=(kb == 0), stop=False)
                            nc.tensor.matmul(
                                cps[:, col0:col0 + 128],
                                v2_s[:, kb, h, :],
                                pt[:, col0:col0 + 128],
                                start=False, stop=last)
                        else:
                            nc.tensor.matmul(
                                cps[:, col0:],
                                v2_s[:, kb, h, :],
                                pt[:, col0:],
                                start=(kb == 0), stop=last)
                        pts[kb] = None
                        if last:
                            rc = rcp.tile([64, QS], F32, tag="rc",
                                          name=f"rc{h}_{qs}")
                            nc.vector.reciprocal(rc[:, :], cps[64:128, :])
                            nc.vector.tensor_mul(
                                ctx_s[po:po + 64, mc, qs * QS:(qs + 1) * QS],
                                cps[0:64, :], rc[:, :])

                    emit_av = emit_av_b if qs > 0 else emit_av_a

                    for kb in range(nkb):
                        st_units.append((lambda kb=kb, f=emit_st: f(kb), 1))
                        av_units.append(lambda kb=kb, f=emit_av: f(kb))

                LEAD = 3
                scored, avd, total = 0, 0, len(av_units)
                while avd < total:
                    if st_units and scored - avd < LEAD:
                        f, k = st_units.pop(0)
                        f()
                        scored += k
                    else:
                        av_units.pop(0)()
                        avd += 1
                    pump(rate)

            # ---------- main schedule ----------
            xt1 = stage_A_dmas(1)
            nc.sync.dma_start(wo_s[:, :, :],
                              wo.rearrange("(c p) n -> p c n", p=128))
            for u, _w in stage_A_units(0, xt0):
                u()
            xts = {1: xt1}

            if "B" in phases:
                for qs in range(NQS):
                    # prefetch x for slice qs+2
                    if qs + 2 < NQS:
                        xts[qs + 2] = stage_A_dmas(qs + 2)
                    # filler: A(qs+1) compute, then C tiles once A is done
                    if qs + 1 < NQS:
                        filler.extend(stage_A_units(qs + 1, xts.pop(qs + 1)))
                    elif "C" in phases:
                        for t in range(12):
                            filler.extend(stage_C_units(t))
                    # driver iterations: st units + av units
                    nsteps = 8 * 2 * (4 * qs + 4)
                    rate = sum(w for _, w in filler) / nsteps + 1e-9
                    emit_B_qs(qs, rate)
                    while filler:
                        filler.pop(0)[0]()

            if "C" in phases:
                t0 = 12 if "B" in phases else 0
                for t in range(t0, NTT):
                    for u, _w in stage_C_units(t, tail=True):
                        u()

            if dbg:
                nc.sync.dma_start(qt_dbg[:, :], qt_s[:, :, :])
                nc.sync.dma_start(kt_dbg[:, :], kt_s[:, :, :])
                nc.sync.dma_start(v2_dbg[:, :], v2_s[:, :, :, :])
                nc.sync.dma_start(ctx_dbg[:, :], ctx_s[:, :, :])

    nc.compile()
    return nc


_NC = None
LAST_RESULTS = None


def _host_maps(inputs):
    bf = ml_dtypes.bfloat16
    f = lambda a: np.asarray(a, dtype=np.float32)
    q, k, v = f(inputs["q"]), f(inputs["k"]), f(inputs["v"])
    wq_w, wq_b = f(inputs["wq_w"]), f(inputs["wq_b"])
    wk_w, wk_b = f(inputs["wk_w"]), f(inputs["wk_b"])
    wv_w, wv_b = f(inputs["wv_w"]), f(inputs["wv_b"])
    wo_w = f(inputs["wo_w"])

    msk = np.ascontiguousarray(
        (np.arange(128)[None, :] >= np.arange(128)[:, None])).astype(bf)

    gmaps = []
    for g in range(2):
        sl = slice(g * GW, (g + 1) * GW)
        gmaps.append(dict(
            wq=np.ascontiguousarray((wq_w[sl] * 0.125).T).astype(bf),
            wk=np.ascontiguousarray(wk_w[sl].T).astype(bf),
            wv=np.ascontiguousarray(wv_w[sl].T).astype(bf),
            wo=np.ascontiguousarray(wo_w[:, sl].T).astype(bf),
            bq=np.ascontiguousarray((wq_b[sl] * 0.125).reshape(4, 128).T),
            bk=np.ascontiguousarray(wk_b[sl].reshape(4, 128).T),
            vb=np.ascontiguousarray(wv_b[sl]),
            msk=msk, idm=np.eye(128, dtype=np.float32).astype(bf)))

    bmaps = []
    for b in range(B):
        bmaps.append(dict(
            xq=np.ascontiguousarray(q[b].T).astype(bf),
            xk=np.ascontiguousarray(k[b].T).astype(bf),
            xv=np.ascontiguousarray(v[b].T).astype(bf)))

    return [dict(**bmaps[c // 2], **gmaps[c % 2]) for c in range(8)]


def kernel(**inputs):
    global _NC, LAST_RESULTS
    import os
    if _NC is None:
        _NC = _build_nc()

    in_maps = _host_maps(inputs)
    trace = bool(int(os.environ.get("KERNEL_TRACE", "0")))
    res = run_bass_kernel_spmd(_NC, in_maps, list(range(8)), trace=trace)
    LAST_RESULTS = res

    wo_b = np.asarray(inputs["wo_b"], dtype=np.float32)
    out = np.empty((B, L, D), np.float32)
    for b in range(B):
        out[b] = (np.asarray(res.results[2 * b]["outp"], np.float32)
                  + np.asarray(res.results[2 * b + 1]["outp"], np.float32)
                  + wo_b[None, :])
    return out


# revision 5
# speedup vs baseline: 1.7645x; 1.0068x over previous
"""Causal MHA (B=4, L=2048, D=1024, H=16) on 8 NeuronCores — v2, all bf16.

Sharding: core c -> (batch b = c//2, head-group g = c%2); 8 heads per core.
Host sums the two head-group partial outputs per batch and adds wo_b.

Per-core structure (single Bass module, software-pipelined emission):
  A(n): per 512-token slice n: QT/KT = w @ x chunks -> psum -> +bias -> SBUF
        bf16 [128, 4, L] (head-dim on partitions).  V2 = x.T @ wv -> psum
        [tok,4,64] -> +bias -> SBUF v2[128 tok, kb, 8 heads, 128] where cols
        64:128 of each head slot are constant 1.0 (preset once): the AV
        matmul then yields the softmax denominator replicated on 64
        partitions for free.
  B(h, qs): score S.T[keys,q] = KT_h.T @ QT_h (causal-trimmed), exp on
        ACT -> pt bf16, tri-mask on diagonal blocks (DVE), AV psum
        [128, q] += v2[kb,h].T @ pt: rows 0:64 ctx, 64:128 denominator.
        ctx = rows/denominator (one DVE divide) -> SBUF ctx bf16.
  C(t): out[tok128, 1024] = sum_c ctx[c].T @ wo[c] -> psum -> bf16 -> DRAM.
  A(n+1) and C(t) matmuls are interleaved into B's emission as filler so
  the PE never waits on the exp (ACT) pipeline.
"""

import numpy as np
import ml_dtypes

import concourse.bacc as bacc
import concourse.bass as bass
import concourse.mybir as mybir
import concourse.tile as tile
from concourse.bass_utils import run_bass_kernel_spmd

F32 = mybir.dt.float32
BF = mybir.dt.bfloat16

B, L, D, H, DK = 4, 2048, 1024, 16, 64
HD = 8            # heads per core
GW = 512          # head-group width (8 heads * 64)
NCH = D // 128    # 8 contraction chunks
QS = 512          # q-slice width in attention
NQS = L // QS     # 4
NKB = L // 128    # 16 key blocks
NTT = L // 128    # 16 token tiles


def _build_nc(dbg=False, phases="ABC"):
    nc = bacc.Bacc("TRN2", target_bir_lowering=False, debug=False, num_devices=8)

    xq = nc.dram_tensor("xq", [D, L], BF, kind="ExternalInput").ap()
    xk = nc.dram_tensor("xk", [D, L], BF, kind="ExternalInput").ap()
    xv = nc.dram_tensor("xv", [D, L], BF, kind="ExternalInput").ap()
    wq = nc.dram_tensor("wq", [D, GW], BF, kind="ExternalInput").ap()
    wk = nc.dram_tensor("wk", [D, GW], BF, kind="ExternalInput").ap()
    wv = nc.dram_tensor("wv", [D, GW], BF, kind="ExternalInput").ap()
    wo = nc.dram_tensor("wo", [GW, D], BF, kind="ExternalInput").ap()
    bq = nc.dram_tensor("bq", [128, 4], F32, kind="ExternalInput").ap()
    bk = nc.dram_tensor("bk", [128, 4], F32, kind="ExternalInput").ap()
    vb = nc.dram_tensor("vb", [GW], F32, kind="ExternalInput").ap()
    msk = nc.dram_tensor("msk", [128, 128], BF, kind="ExternalInput").ap()
    idm = nc.dram_tensor("idm", [128, 128], BF, kind="ExternalInput").ap()
    outp = nc.dram_tensor("outp", [L, D], BF, kind="ExternalOutput").ap()
    if dbg:
        qt_dbg = nc.dram_tensor("qt_dbg", [128, 4 * L], BF, kind="ExternalOutput").ap()
        kt_dbg = nc.dram_tensor("kt_dbg", [128, 4 * L], BF, kind="ExternalOutput").ap()
        v2_dbg = nc.dram_tensor("v2_dbg", [128, NKB * HD * 128], BF,
                                kind="ExternalOutput").ap()
        ctx_dbg = nc.dram_tensor("ctx_dbg", [128, 4 * L], BF, kind="ExternalOutput").ap()

    with tile.TileContext(nc) as tc:
        with (
            tc.tile_pool(name="persist", bufs=1) as persist,
            tc.tile_pool(name="xin", bufs=3) as xinp,
            tc.tile_pool(name="pt", bufs=10) as ptp,
            tc.tile_pool(name="outs", bufs=6) as outsp,
            tc.tile_pool(name="psQK", bufs=2, space="PSUM") as psQK,
            tc.tile_pool(name="cb", bufs=4) as cbp,
            tc.tile_pool(name="rc", bufs=2) as rcp,
            tc.tile_pool(name="psS", bufs=3, space="PSUM") as psS,
            tc.tile_pool(name="psT", bufs=1, space="PSUM") as psT,
            tc.tile_pool(name="psAV", bufs=2, space="PSUM") as psAV,
        ):
            # ---- persistent SBUF ----
            wq_s = persist.tile([128, NCH, GW], BF, tag="wq")
            wk_s = persist.tile([128, NCH, GW], BF, tag="wk")
            wv_s = persist.tile([128, NCH, GW], BF, tag="wv")
            wo_s = persist.tile([128, 4, D], BF, tag="wo")
            qt_s = persist.tile([128, 4, L], BF, tag="qt")
            kt_s = persist.tile([128, 4, L], BF, tag="kt")
            ctx_s = persist.tile([128, 4, L], BF, tag="ctx")
            v2_s = persist.tile([128, NKB, HD, 128], BF, tag="v2")
            bq_s = persist.tile([128, 4], F32, tag="bq")
            bk_s = persist.tile([128, 4], F32, tag="bk")
            vb_s = persist.tile([128, 2, 4, DK], F32, tag="vb")
            msk_s = persist.tile([128, 128], BF, tag="msk")
            idm_s = persist.tile([128, 128], BF, tag="idm")

            # ---------- phase A emission helpers ----------
            def stage_A_dmas(n, split=False):
                """Issue the x DMAs for token-slice n; return the SBUF tiles."""
                xt = {}
                for nm, src in (("q", xq), ("k", xk), ("v", xv)):
                    t = xinp.tile([128, NCH, QS], BF, tag=f"x{nm}",
                                  name=f"x{nm}{n}")
                    halves = ((0, 4), (4, 8)) if split else ((0, 8),)
                    for c0, c1 in halves:
                        nc.sync.dma_start(
                            t[:, c0:c1, :],
                            src[c0 * 128:c1 * 128,
                                n * QS:(n + 1) * QS].rearrange(
                                "(c p) t -> p c t", p=128))
                    xt[nm] = t
                return xt

            # startup: interleave first-slice x loads with their weights in
            # chunk-halves so the first matmuls start after ~2 half loads
            def _wload(dst, src, c0, c1):
                nc.sync.dma_start(
                    dst[:, c0:c1, :],
                    src[c0 * 128:c1 * 128, :].rearrange("(c p) n -> p c n", p=128))

            vb_bcast = bass.AP(tensor=vb.tensor, offset=vb.offset,
                               ap=[[0, 128], [256, 2], [DK, 4], [1, DK]])
            nc.gpsimd.dma_start(vb_s[:, :, :, :], vb_bcast)
            # ones-columns of v2 (softmax denominator trick), preset once
            nc.gpsimd.memset(v2_s[:, :, :, DK:128], 1.0)

            xt0 = {}
            first = True
            for nm, xsrc, wdst, wsrc in (("q", xq, wq_s, wq),
                                         ("k", xk, wk_s, wk),
                                         ("v", xv, wv_s, wv)):
                t = xinp.tile([128, NCH, QS], BF, tag=f"x{nm}", name=f"x{nm}0")
                quarters = ((0, 4), (4, 8))
                for c0, c1 in quarters:
                    _wload(wdst, wsrc, c0, c1)
                    nc.sync.dma_start(
                        t[:, c0:c1, :],
                        xsrc[c0 * 128:c1 * 128, 0:QS].rearrange(
                            "(c p) t -> p c t", p=128))
                    if first:
                        # tiny constant loads right after the first quarter —
                        # bias-adds gate PSUM slot recycling, but the very
                        # first matmuls only need wq/xq chunks 0:2
                        nc.sync.dma_start(bq_s[:, :], bq[:, :])
                        nc.sync.dma_start(bk_s[:, :], bk[:, :])
                        nc.sync.dma_start(msk_s[:, :], msk[:, :])
                        nc.sync.dma_start(idm_s[:, :], idm[:, :])
                        first = False
                xt0[nm] = t

            def stage_A_units(n, xt, qk_only=False):
                """Compute units (closures) for token-slice n."""
                units = []
                for (nm, w_s, dst, b_s) in (("q", wq_s, qt_s, bq_s),
                                            ("k", wk_s, kt_s, bk_s)):
                    xti = xt[nm]
                    mms, fins = [], []
                    for m in range(4):
                        ps = psQK.tile([128, QS], F32, tag="ps",
                                       name=f"psA_{nm}{n}_{m}")

                        def mm(c0, c1, nm=nm, m=m, ps=ps, xti=xti, w_s=w_s):
                            for c in range(c0, c1):
                                nc.tensor.matmul(
                                    ps[:, :],
                                    w_s[:, c, m * 128:(m + 1) * 128],
                                    xti[:, c, :],
                                    start=(c == 0), stop=(c == NCH - 1))

                        def fin(nm=nm, m=m, n=n, ps=ps, dst=dst, b_s=b_s):
                            nc.vector.tensor_scalar_add(
                                dst[:, m, n * QS:(n + 1) * QS],
                                ps[:, :], b_s[:, m:m + 1])

                        mms.append(mm)
                        fins.append(fin)
                    # pair m-tiles: both first-halves, then both second-halves
                    # (+bias) — lets compute start when only the first chunks
                    # of the weights/inputs have landed (startup split loads)
                    if True:
                        for m0 in (0, 2):
                            units.append((lambda mm=mms[m0]: mm(0, 4), 853))
                            units.append((lambda mm=mms[m0 + 1]: mm(0, 4), 853))
                            units.append((lambda mm=mms[m0], fin=fins[m0]:
                                          (mm(4, 8), fin()), 853))
                            units.append((lambda mm=mms[m0 + 1],
                                          fin=fins[m0 + 1]:
                                          (mm(4, 8), fin()), 853))
                if qk_only:
                    return units
                units.extend(stage_V_units(n, xt))
                return units

            def stage_V_units(n, xt):
                units = []
                # V: per token-tile tt (128 tokens), half hf (4 heads)
                xtv = xt["v"]
                for tt in range(4):
                    tg = n * 4 + tt
                    for hf in range(2):
                        ps = psQK.tile([128, QS], F32, tag="ps",
                                       name=f"psV{tg}_{hf}")

                        def mmv(c0, tt=tt, hf=hf, ps=ps, xtv=xtv):
                            for c in range(c0, c0 + 4):
                                nc.tensor.matmul(
                                    ps[:, 0:256],
                                    xtv[:, c, tt * 128:(tt + 1) * 128],
                                    wv_s[:, c, hf * 256:(hf + 1) * 256],
                                    start=(c == 0), stop=(c == NCH - 1))

                        def finv(tg=tg, hf=hf, ps=ps):
                            nc.vector.tensor_add(
                                v2_s[:, tg, hf * 4:(hf + 1) * 4, 0:DK],
                                ps[:, 0:256], vb_s[:, hf, :, :])

                        units.append((lambda mmv=mmv: mmv(0), 427))
                        units.append((lambda mmv=mmv, finv=finv:
                                      (mmv(4), finv()), 427))
                return units

            # ---------- phase C emission helpers ----------
            def stage_C_units(t, tail=False):
                units = []
                # in the tail, B is done: borrow the score pool (3 bufs) so
                # psum slot recycling never gates the matmuls
                pool, tg = (psS, "sp") if tail else (psQK, "ps")
                for n2 in range(2):
                    ps = pool.tile([128, QS], F32, tag=tg,
                                   name=f"psC{t}_{n2}")

                    def mmc(ps=ps, t=t, n2=n2):
                        for c in range(4):
                            nc.tensor.matmul(
                                ps[:, :],
                                ctx_s[:, c, t * 128:(t + 1) * 128],
                                wo_s[:, c, n2 * QS:(n2 + 1) * QS],
                                start=(c == 0), stop=(c == 3))

                    def finc(ps=ps, t=t, n2=n2, tail=tail):
                        ot = outsp.tile([128, QS], BF, tag="ot",
                                        name=f"ot{t}_{n2}")
                        # scalar engine is idle in the tail; DVE during B
                        if tail:
                            nc.scalar.copy(ot[:, :], ps[:, :])
                        else:
                            nc.vector.tensor_copy(ot[:, :], ps[:, :])
                        nc.sync.dma_start(
                            outp[t * 128:(t + 1) * 128,
                                 n2 * QS:(n2 + 1) * QS], ot[:, :])

                    units.append((mmc, 853))
                    units.append((finc, 0))
                return units

            # ---------- filler pump ----------
            filler = []
            pump_acc = [0.0]

            def pump(budget_ns):
                pump_acc[0] += budget_ns
                while filler and pump_acc[0] >= filler[0][1]:
                    fn, w = filler.pop(0)
                    fn()
                    pump_acc[0] -= w
                if not filler:
                    pump_acc[0] = 0.0

            # ---------- phase B ----------
            # Global score->exp->AV software pipeline per q-slice, hybrid AV:
            #  - qs=0: classic orientation (moving dim = q) — the V/A(1)
            #    filler makes qs0 PE-paced, heavy AVs act as their own filler.
            #  - qs>=1: flipped orientation cps2[q-part, qt*65:(qt+1)*65] +=
            #    P.T @ V (moving dim 65): ~56k fewer PE cycles; column 64 is
            #    the softmax denominator (ones-column of v2_s); per-partition
            #    tensor-scalar divide + PE transpose restore [hd, tok].
            # PSUM pending-zero discipline: exactly ONE start=True (first
            # write of kb=0 — it touches every region of the bank) and ONE
            # stop=True (last write) per cps accumulation bank.
            def emit_B_qs(qs, rate):
                nkb = 4 * qs + 4
                st_units, av_units = [], []
                for h in range(HD):
                    po = (h % 2) * 64
                    mc = h // 2
                    cpsd = {}
                    pts = [None] * nkb

                    def score_mm(sp, kb, col0, po=po, mc=mc):
                        nc.tensor.matmul(
                            sp[:, col0:],
                            kt_s[po:po + 64, mc, kb * 128:(kb + 1) * 128],
                            qt_s[po:po + 64, mc, qs * QS + col0:(qs + 1) * QS],
                            start=True, stop=True)

                    def emit_st(kb, h=h, pts=pts, smm=score_mm):
                        col0 = max(0, kb * 128 - qs * QS)
                        sp = psS.tile([128, QS], F32, tag="sp",
                                      name=f"sp{h}_{qs}_{kb}")
                        smm(sp, kb, col0)
                        pt = ptp.tile([128, QS], BF, tag="pt",
                                      name=f"pt{h}_{qs}_{kb}")
                        nc.scalar.activation(
                            pt[:, col0:], sp[:, col0:],
                            func=mybir.ActivationFunctionType.Exp)
                        if col0 > 0 or kb == 4 * qs:
                            nc.vector.tensor_mul(
                                pt[:, col0:col0 + 128],
                                pt[:, col0:col0 + 128], msk_s[:, :])
                        pts[kb] = pt

                    def emit_av_b(kb, h=h, po=po, mc=mc, pts=pts, cpsd=cpsd):
                        if kb == 0:
                            cpsd["t"] = psAV.tile([128, QS], F32, tag="cps",
                                                  name=f"cps{h}_{qs}")
                        cps2 = cpsd["t"]
                        qt0 = max(0, kb - 4 * qs)
                        pt = pts[kb]
                        # masked qt0 tile last so unmasked AVs follow exp
                        order = list(range(qt0 + 1, 4)) + [qt0]
                        for i, qt in enumerate(order):
                            nc.tensor.matmul(
                                cps2[:, qt * 65:(qt + 1) * 65],
                                pt[:, qt * 128:(qt + 1) * 128],
                                v2_s[:, kb, h, 0:DK + 1],
                                start=(kb == 0 and i == 0),
                                stop=(kb == nkb - 1 and qt == qt0))
                        pts[kb] = None
                        if kb == nkb - 1:
                            # bank-wide stop just fired: normalize all four
                            # q-tiles, transpose each to [hd, tok], store
                            for qt in range(4):
                                ctxB = cbp.tile([128, DK], BF, tag="cb",
                                                name=f"cb{h}_{qs}_{qt}")
                                rc2 = rcp.tile([128, 4], F32, tag="rcb",
                                               name=f"rcb{h}_{qs}_{qt}")
                                nc.vector.reciprocal(
                                    rc2[:, 0:1],
                                    cps2[:, qt * 65 + DK:qt * 65 + DK + 1])
                                nc.vector.tensor_scalar_mul(
                                    ctxB[:, :], cps2[:, qt * 65:qt * 65 + DK],
                                    rc2[:, 0:1])
                                tr = psT.tile([DK, 128], BF, tag="tr",
                                              name=f"tr{h}_{qs}_{qt}")
                                nc.tensor.matmul(tr[:, :], ctxB[:, :],
                                                 idm_s[:, :],
                                                 is_transpose=True)
                                nc.vector.tensor_copy(
                                    ctx_s[po:po + 64, mc,
                                          qs * QS + qt * 128:
                                          qs * QS + (qt + 1) * 128],
                                    tr[:, :])

                    def emit_av_a(kb, h=h, po=po, mc=mc, pts=pts, cpsd=cpsd):
                        if kb == 0:
                            cpsd["t"] = psAV.tile([128, QS], F32, tag="cps",
                                                  name=f"cps{h}_{qs}")
                        cps = cpsd["t"]
                        col0 = max(0, kb * 128 - qs * QS)
                        pt = pts[kb]
                        last = kb == nkb - 1
                        if col0 > 0 or kb == 4 * qs:
                            if col0 + 128 < QS:
                                nc.tensor.matmul(
                                    cps[:, col0 + 128:],
                                    v2_s[:, kb, h, :],
                                    pt[:, col0 + 128:],
                                    start=(kb == 0), stop=False)
                            nc.tensor.matmul(
                                cps[:, col0:col0 + 128],
                                v2_s[:, kb, h, :],
                                pt[:, col0:col0 + 128],
                                start=False, stop=last)
                        else:
                            nc.tensor.matmul(
                                cps[:, col0:],
                                v2_s[:, kb, h, :],
                                pt[:, col0:],
                                start=(kb == 0), stop=last)
                        pts[kb] = None
                        if last:
                            rc = rcp.tile([64, QS], F32, tag="rc",
                                          name=f"rc{h}_{qs}")
                            nc.vector.reciprocal(rc[:, :], cps[64:128, :])
                            nc.vector.tensor_mul(
                                ctx_s[po:po + 64, mc, qs * QS:(qs + 1) * QS],
                                cps[0:64, :], rc[:, :])

                    emit_av = emit_av_b if qs > 0 else emit_av_a

                    for kb in range(nkb):
                        st_units.append((lambda kb=kb, f=emit_st: f(kb), 1))
                        av_units.append(lambda kb=kb, f=emit_av: f(kb))

                LEAD = 6
                scored, avd, total = 0, 0, len(av_units)
                while avd < total:
                    if st_units and scored - avd < LEAD:
                        f, k = st_units.pop(0)
                        f()
                        scored += k
                    else:
                        av_units.pop(0)()
                        avd += 1
                    pump(rate)

            # ---------- main schedule ----------
            xt1 = stage_A_dmas(1)
            nc.sync.dma_start(wo_s[:, :, :],
                              wo.rearrange("(c p) n -> p c n", p=128))
            for u, _w in stage_A_units(0, xt0):
                u()
            xts = {1: xt1}

            if "B" in phases:
                for qs in range(NQS):
                    # prefetch x for slice qs+2
                    if qs + 2 < NQS:
                        xts[qs + 2] = stage_A_dmas(qs + 2)
                    # filler: A(qs+1) compute, then C tiles once A is done
                    if qs + 1 < NQS:
                        filler.extend(stage_A_units(qs + 1, xts.pop(qs + 1)))
                    elif "C" in phases:
                        for t in range(12):
                            filler.extend(stage_C_units(t))
                    # driver iterations: st units + av units
                    nsteps = 8 * 2 * (4 * qs + 4)
                    rate = sum(w for _, w in filler) / nsteps + 1e-9
                    emit_B_qs(qs, rate)
                    while filler:
                        filler.pop(0)[0]()

            if "C" in phases:
                t0 = 12 if "B" in phases else 0
                for t in range(t0, NTT):
                    for u, _w in stage_C_units(t, tail=True):
                        u()

            if dbg:
                nc.sync.dma_start(qt_dbg[:, :], qt_s[:, :, :])
                nc.sync.dma_start(kt_dbg[:, :], kt_s[:, :, :])
                nc.sync.dma_start(v2_dbg[:, :], v2_s[:, :, :, :])
                nc.sync.dma_start(ctx_dbg[:, :], ctx_s[:, :, :])

    nc.compile()
    return nc


_NC = None
LAST_RESULTS = None


def _host_maps(inputs):
    bf = ml_dtypes.bfloat16
    f = lambda a: np.asarray(a, dtype=np.float32)
    q, k, v = f(inputs["q"]), f(inputs["k"]), f(inputs["v"])
    wq_w, wq_b = f(inputs["wq_w"]), f(inputs["wq_b"])
    wk_w, wk_b = f(inputs["wk_w"]), f(inputs["wk_b"])
    wv_w, wv_b = f(inputs["wv_w"]), f(inputs["wv_b"])
    wo_w = f(inputs["wo_w"])

    msk = np.ascontiguousarray(
        (np.arange(128)[None, :] >= np.arange(128)[:, None])).astype(bf)

    gmaps = []
    for g in range(2):
        sl = slice(g * GW, (g + 1) * GW)
        gmaps.append(dict(
            wq=np.ascontiguousarray((wq_w[sl] * 0.125).T).astype(bf),
            wk=np.ascontiguousarray(wk_w[sl].T).astype(bf),
            wv=np.ascontiguousarray(wv_w[sl].T).astype(bf),
            wo=np.ascontiguousarray(wo_w[:, sl].T).astype(bf),
            bq=np.ascontiguousarray((wq_b[sl] * 0.125).reshape(4, 128).T),
            bk=np.ascontiguousarray(wk_b[sl].reshape(4, 128).T),
            vb=np.ascontiguousarray(wv_b[sl]),
            msk=msk, idm=np.eye(128, dtype=np.float32).astype(bf)))

    bmaps = []
    for b in range(B):
        bmaps.append(dict(
            xq=np.ascontiguousarray(q[b].T).astype(bf),
            xk=np.ascontiguousarray(k[b].T).astype(bf),
            xv=np.ascontiguousarray(v[b].T).astype(bf)))

    return [dict(**bmaps[c // 2], **gmaps[c % 2]) for c in range(8)]


def kernel(**inputs):
    global _NC, LAST_RESULTS
    import os
    if _NC is None:
        _NC = _build_nc()

    in_maps = _host_maps(inputs)
    trace = bool(int(os.environ.get("KERNEL_TRACE", "0")))
    res = run_bass_kernel_spmd(_NC, in_maps, list(range(8)), trace=trace)
    LAST_RESULTS = res

    wo_b = np.asarray(inputs["wo_b"], dtype=np.float32)
    out = np.empty((B, L, D), np.float32)
    for b in range(B):
        out[b] = (np.asarray(res.results[2 * b]["outp"], np.float32)
                  + np.asarray(res.results[2 * b + 1]["outp"], np.float32)
                  + wo_b[None, :])
    return out
